# revision 48
# baseline (speedup 1.0000x reference)
"""Trainium2 Bass kernel for nn_ButterflyLayer2D (butterfly 2D CNN).

Strategy: pure data parallel over 8 NeuronCores (16 batch each).

Layout invariant: activations live in SBUF as [128 = (w%2)*64 + c,
(node, b, h, w//2)] bf16.  Each per-node 2x2-stride-2 conv is then a set
of matmuls: contraction K=128=(dw,c), accumulated over x=dh (start/stop),
with output w-parity q selected by slicing the rhs w2 dim (q::2) and
writing psum rows q*64+c via col tile_position (0, q*64).  The psum rows
(q, c_out) are exactly the next level's partition layout, so every
eviction is a contiguous full-width relu+bias op.

PSUM tiles are [128, 1024] (2 banks, ring of 4).  Within a tile the
matmuls are ordered x-outer so one weight image (w[n,x] in both column
halves) serves 2 consecutive 512-col streams.  Each tile is drained by
ONE [128, 1024] relu+bias op (the empirically fastest evict shape,
~730ns), alternating between ScalarE and VectorE per tile.  X is split
into 8 bl-pair tiles so L1 matmuls depend only on the input-conv
evictions of their own chunk (tile-granular dependency tracking).

When a level's bias is nonzero, multi-node tiles fall back to per-node
evictions (bias must be per-partition-constant within one evict op).
"""

import numpy as np
from contextlib import ExitStack

import concourse.bass as bass
import concourse.tile as tile
from concourse import bacc, mybir
from concourse.bass_utils import run_bass_kernel_spmd

F32 = mybir.dt.float32
BF16 = mybir.dt.bfloat16
AF = mybir.ActivationFunctionType
ALU = mybir.AluOpType

B, IN, NLVL, KLVL, C = 128, 256, 6, 3, 64
NK, OU, OV = 8, 8, 8
NCORES = 8
BC = B // NCORES          # 16 per-core batch
BG = BC
HALF = BG // 4
LVL_NODES = [4, 16, 64, 64, 64, 64]
LVL_HIN = [64, 32, 16, 8, 4, 2]
WCH = 16                  # weight streaming chunk (nodes)
BIAS_OFF = {0: 0, 1: 1, 2: 5, 3: 21, 4: 85, 5: 149, 6: 213}  # cols in bias blob


# ----------------------------------------------------------------------------
# host-side pre-arrangement
# ----------------------------------------------------------------------------

def _prep_weights(inputs):
    """Weights/biases blobs shared by all cores."""
    import ml_dtypes

    out = {}
    zflags = {}
    # fin2 [128=(G,p,qp,yp), 128=(q,c)]: W = fin[p,qp,c] iff yp==q, else 0.
    # K=32 per batch-group G; the zero rows select output w-parity q so one
    # M=128 matmul produces both parity halves of the psum tile.
    fin = inputs["in_filter"][:, :, 0, :].astype(np.float32)  # [p, qp, c]
    f2 = np.zeros((4, 4, 2, 2, C), np.float32)                # [p,qp,yp,q,c]
    for q in range(2):
        f2[:, :, q, q, :] = fin
    f2 = f2.reshape(32, 2 * C)
    finr = np.zeros((128, 2 * C), np.float32)
    for g in range(4):
        finr[g * 32 : (g + 1) * 32] = f2
    out["fin"] = finr.astype(ml_dtypes.bfloat16)

    bias_cols = [np.concatenate([inputs["in_bias"], inputs["in_bias"]])
                 .reshape(128, 1).astype(np.float32)]
    for lvl in range(1, NLVL + 1):
        f = inputs[f"f{lvl}"].astype(np.float32)  # [n,n,2,2,C,C] (x=dh,y=dw,ci,co)
        n = f.shape[0]
        assert n == 2 ** min(lvl, KLVL)
        w = f.transpose(0, 1, 3, 4, 2, 5).reshape(n * n, 2 * C, 2 * C)
        out[f"w{lvl}"] = np.ascontiguousarray(w.transpose(1, 0, 2)).reshape(
            128, n * n * 128
        ).astype(ml_dtypes.bfloat16)
        b = inputs[f"b{lvl}"].astype(np.float32).reshape(n * n, C)
        zflags[lvl] = not np.any(b)
        if lvl < NLVL:
            bb = np.concatenate([b, b], axis=1)  # rows (q,c), dup across q
            bias_cols.append(np.ascontiguousarray(bb.T))
        else:
            bb = b.reshape(n * n // 2, 2 * C)    # rows (cEven,cOdd) per pair
            bias_cols.append(np.ascontiguousarray(bb.T))
    out["bb"] = np.concatenate(bias_cols, axis=1)  # [128, 245]

    # dense, stacked pairs: rows 0:64 = c for even node, 64:128 = odd;
    # cols (pair, (r,ou,ov))
    wd = inputs["Wd"].astype(np.float32).reshape(NK * NK, 2, C, OU * OV)
    wdn = wd.transpose(0, 2, 1, 3).reshape(NK * NK, C, 2 * OU * OV)
    wds = np.zeros((128, (NK * NK // 2) * 2 * OU * OV), np.float32)
    for p in range(NK * NK // 2):
        wds[0:64, p * 128 : (p + 1) * 128] = wdn[2 * p]
        wds[64:128, p * 128 : (p + 1) * 128] = wdn[2 * p + 1]
    out["wd"] = np.ascontiguousarray(wds).astype(ml_dtypes.bfloat16)
    return out, zflags


def _prep_input(in_data_core):
    """Per-core input blob [128 = (b//4)*32 + (i%4)*8 + (j%4)*2 + (j//4)%2,
    (b%4, x=i//4, y2=j//8)] — all 128 partitions so the staging DMA runs
    at full width; column chunks (by b%4) land in consumption order."""
    import ml_dtypes

    ind = in_data_core[:, :, :, 0]  # [16, 256, 256]
    a = ind.reshape(4, 4, 64, 4, 32, 2, 4)      # [G, m, x, p, y2, yp, qp]
    a = a.transpose(0, 3, 6, 5, 1, 2, 4)        # [G, p, qp, yp, m, x, y2]
    return np.ascontiguousarray(a).reshape(128, 4 * 64 * 32).astype(
        ml_dtypes.bfloat16
    )


def _decode_output(t2_core):
    """t2 [128=(r,ou,ov), (par, pair, bl)], node=2*pair+par -> [16, 64, 64, 2]."""
    t = t2_core.reshape(2, OU, OV, 2, 32, BG)       # r,ou,ov,par,pair,bl
    t = t.transpose(0, 1, 2, 4, 3, 5)               # r,ou,ov,pair,par,bl
    t = t.reshape(2, OU, OV, NK, NK, BG)            # r,ou,ov,u,v,bl
    t = t.transpose(5, 3, 1, 4, 2, 0)               # bl,u,ou,v,ov,r
    return np.ascontiguousarray(t).reshape(BG, NK * OU, NK * OV, 2)


# ----------------------------------------------------------------------------
# device kernel
# ----------------------------------------------------------------------------

def _build_kernel(zflags, debug=False):
    nc = bacc.Bacc(None, target_bir_lowering=False)
    p = {}
    p["a0"] = nc.declare_dram_parameter("a0", [128, 4 * 64 * 32], BF16, isOutput=False)
    p["fin"] = nc.declare_dram_parameter("fin", [128, 2 * C], BF16, isOutput=False)
    p["bb"] = nc.declare_dram_parameter("bb", [128, 245], F32, isOutput=False)
    for lvl in range(1, NLVL + 1):
        n2 = LVL_NODES[lvl - 1]
        p[f"w{lvl}"] = nc.declare_dram_parameter(f"w{lvl}", [128, n2 * 128], BF16, isOutput=False)
    p["wd"] = nc.declare_dram_parameter("wd", [128, 32 * 128], BF16, isOutput=False)
    t2 = nc.declare_dram_parameter("t2", [128, NK * NK * BG], F32, isOutput=True)
    dbg = {}
    if debug:
        dbg["X"] = nc.declare_dram_parameter("dbgX", [128, BG * 64 * 32], BF16, isOutput=True)
        for lvl in range(1, 6):
            n2 = LVL_NODES[lvl - 1]
            Ho = LVL_HIN[lvl - 1] // 2
            dbg[lvl] = nc.declare_dram_parameter(
                f"dbgL{lvl}", [128, n2 * BG * Ho * max(Ho // 2, 1)], BF16, isOutput=True)
        dbg["F"] = nc.declare_dram_parameter("dbgF", [128, 32 * BG], BF16, isOutput=True)

    evict_ctr = [0]

    with tile.TileContext(nc) as tc, ExitStack() as ctx:
        const = ctx.enter_context(tc.tile_pool(name="const", bufs=1))
        wpool = ctx.enter_context(tc.tile_pool(name="wts", bufs=8))
        apool = ctx.enter_context(tc.tile_pool(name="acts", bufs=1))
        inpool = ctx.enter_context(tc.tile_pool(name="inp", bufs=1))
        fpool = ctx.enter_context(tc.tile_pool(name="feat", bufs=1))
        ppool = ctx.enter_context(tc.tile_pool(name="ps", bufs=4, space="PSUM"))

        # --- startup DMAs: fin, input chunks, bias blob, then weights ---
        fin_t = const.tile([128, 2 * C], BF16)
        nc.sync.dma_start(fin_t[:], p["fin"][:])
        a0m = [inpool.tile([128, 64 * 32], BF16, tag=f"a0s{m}", name=f"a0s{m}")
               for m in range(4)]
        nc.gpsimd.dma_start(a0m[0][:, 0:1024], p["a0"][:, 0:1024])
        bb_t = const.tile([128, 245], F32, tag="bb", name="bb")
        nc.sync.dma_start(bb_t[:], p["bb"][:])
        nc.gpsimd.dma_start(a0m[0][:, 1024:2048], p["a0"][:, 1024:2048])
        for m in range(1, 4):
            for h in range(2):
                nc.gpsimd.dma_start(
                    a0m[m][:, h * 1024 : (h + 1) * 1024],
                    p["a0"][:, m * 2048 + h * 1024 : m * 2048 + (h + 1) * 1024],
                )
        a0v = [t_[:].rearrange("p (x y) -> p x y", x=64) for t_ in a0m]

        # PE warm-up: dummy matmuls on an uninitialized junk tile (no DMA
        # dependency) into a dedicated 1-bank psum tile nobody reads, so the
        # HAM clock gate reaches 8/8 before real work starts and stays there
        # through the input-chunk staircase.
        junk = const.tile([128, 128], BF16, tag="junk", name="junk")
        nc.vector.memset(junk[:], 1.0)
        warm = ppool.tile([128, 128], F32, tag="ps", name="warm")

        def dummies(k):
            for _ in range(k):
                nc.tensor.matmul(
                    warm[:], junk[:], junk[:], start=True, stop=True,
                )

        dummies(40)

        def bias_ap(lvl, n):
            off = BIAS_OFF[lvl] + n
            return bb_t[:, off : off + 1]

        def evict(out_ap, psum_ap, b_ap):
            """relu(psum + bias) -> sbuf, alternating engines per op."""
            evict_ctr[0] += 1
            if evict_ctr[0] % 2 == 0:
                if b_ap is None:
                    nc.scalar.activation(out_ap, psum_ap, AF.Relu)
                else:
                    nc.scalar.activation(out_ap, psum_ap, AF.Relu, bias=b_ap)
            else:
                if b_ap is None:
                    nc.vector.tensor_scalar(out_ap, psum_ap, 0.0, None, op0=ALU.max)
                else:
                    nc.vector.tensor_scalar(out_ap, psum_ap, b_ap, 0.0,
                                            op0=ALU.add, op1=ALU.max)

        def wchunk(lvl, g0, gn):
            wlt = wpool.tile([128, WCH * 128], BF16, tag="wch", name=f"w{lvl}_{g0}")
            src = p["wd"] if lvl == "d" else p[f"w{lvl}"]
            nc.gpsimd.dma_start(
                wlt[:, : gn * 128], src[:, g0 * 128 : (g0 + gn) * 128]
            )
            return wlt

        # ---------------- input conv + L1, interleaved ----------------
        # X split into 8 bl-pair tiles: [128=(w%2,c), (bl2, h=64, w2=32)]
        Xtile, Xt = [], []
        for u in range(8):
            t_ = apool.tile([128, 2 * 64 * 32], BF16, tag=f"sA{u}", name=f"x{u}")
            Xtile.append(t_)
            Xt.append(t_[:].rearrange("p (b h w) -> p b h w", b=2, h=64))
        L1n = LVL_NODES[0]
        L1out = apool.tile([128, L1n * BG * 32 * 16], BF16, tag="sB", name="a1")
        L1v = L1out[:].rearrange("p (n b h w) -> p n b h w", n=L1n, b=BG, h=32)
        w1t = wchunk(1, 0, L1n)

        def input_bl(bl):
            g, m = bl // 4, bl % 4
            for th in (0, 1):
                pt = ppool.tile([128, 1024], F32, tag="ps", name=f"pin{bl}_{th}")
                for j in (0, 1):
                    xq = th * 2 + j
                    nc.tensor.matmul(
                        pt[:, j * 512 : (j + 1) * 512],
                        fin_t[g * 32 : (g + 1) * 32, :],
                        a0v[m][g * 32 : (g + 1) * 32,
                               xq * 16 : (xq + 1) * 16, :],
                        start=True, stop=True,
                        tile_position=(g * 32, 0),
                    )
                evict(Xt[bl // 2][:, bl % 2, th * 32 : (th + 1) * 32, :],
                      pt[:], bias_ap(0, 0))

        def l1_pair(n, ua, ub):
            # one node, two bl-pair tiles, x-outer across both (one weight
            # image per x-phase serves 4 streams)
            pts = {ua: ppool.tile([128, 1024], F32, tag="ps", name=f"p1_{n}_{ua}"),
                   ub: ppool.tile([128, 1024], F32, tag="ps", name=f"p1_{n}_{ub}")}
            for x in (0, 1):
                for u in (ua, ub):
                    for j in (0, 1):
                        for q in (0, 1):
                            nc.tensor.matmul(
                                pts[u][q * 64 : (q + 1) * 64,
                                       j * 512 : (j + 1) * 512],
                                w1t[:, n * 128 + x * 64 : n * 128 + (x + 1) * 64],
                                Xt[u][:, j, x::2, q::2],
                                start=(x == 0), stop=(x == 1),
                                skip_group_check=True,
                                tile_position=(0, q * 64),
                            )
            for u in (ua, ub):
                evict(L1v[:, n, 2 * u : 2 * u + 2, :, :], pts[u][:],
                      bias_ap(1, n))

        for G in range(4):
            input_bl(G * 4)
        for G in range(4):
            input_bl(G * 4 + 1)
        for G in range(4):
            input_bl(G * 4 + 2)
            l1_pair(G, 0, 2)
        for G in range(4):
            input_bl(G * 4 + 3)
            l1_pair(G, 4, 6)
        for n in range(L1n):
            l1_pair(n, 1, 3)
        for n in range(L1n):
            l1_pair(n, 5, 7)
        if debug:
            for u in range(8):
                nc.sync.dma_start(
                    dbg["X"][:, u * 4096 : (u + 1) * 4096], Xtile[u][:]
                )
            nc.sync.dma_start(dbg[1][:], L1out[:])

        # ---------------- levels 2..5 ----------------
        # L2 output is segmented into 8 node-pair tiles reusing the X chunk
        # tags (finer deps + SBUF reuse); L3 reuses sB after L1out dies;
        # L4/L5 outputs reuse sA0/sA1.
        cur_segs = [(L1v, L1n)]
        for lvl in range(2, 6):
            n2 = LVL_NODES[lvl - 1]
            grid = int(np.sqrt(n2))
            Hin = LVL_HIN[lvl - 1]
            Ho, W2o = Hin // 2, Hin // 4
            pcols = BG * Ho * W2o               # output cols per node
            npt = max(1, 1024 // pcols)         # nodes per psum tile
            zb = zflags[lvl]
            pgrid = int(np.sqrt(LVL_NODES[lvl - 2] if lvl > 2 else 1))
            pgrid = int(np.sqrt(cur_segs[0][1] * len(cur_segs)))

            def parent(n):
                if lvl <= KLVL:
                    return (n // grid // 2) * pgrid + (n % grid) // 2
                return n

            def cur_ap(pn):
                nps = cur_segs[0][1]
                return cur_segs[pn // nps][0], pn % nps

            if lvl == 2:
                otiles = [apool.tile([128, 2 * BG * Ho * W2o], BF16,
                                     tag=f"sA{u}", name=f"a2_{u}")
                          for u in range(8)]
                out_segs = [(t_[:].rearrange("p (n b h w) -> p n b h w",
                                             n=2, b=BG, h=Ho), 2)
                            for t_ in otiles]
            else:
                tag = {3: "sB", 4: "sA0", 5: "sA1"}[lvl]
                ot = apool.tile([128, n2 * BG * Ho * W2o], BF16,
                                tag=tag, name=f"a{lvl}")
                otiles = [ot]
                out_segs = [(ot[:].rearrange("p (n b h w) -> p n b h w",
                                             n=n2, b=BG, h=Ho), n2)]

            def nxt_ap(n):
                nps = out_segs[0][1]
                return out_segs[n // nps][0], n % nps

            if pcols >= 1024:
                # one or more [128,1024] tiles per node; 512-col b-blocks
                tpn = pcols // 1024             # tiles per node
                bpt = BG // tpn                 # b per tile
                bpb = max(1, bpt // 2)          # b per 512-block
                for g0 in range(0, n2, WCH):
                    wlt = wchunk(lvl, g0, min(WCH, n2 - g0))
                    for n in range(g0, g0 + min(WCH, n2 - g0)):
                        ln = n - g0
                        cv, lpn = cur_ap(parent(n))
                        nv, lon = nxt_ap(n)
                        for tt in range(tpn):
                            pt = ppool.tile([128, 1024], F32, tag="ps",
                                            name=f"p{lvl}_{n}_{tt}")
                            for x in (0, 1):
                                for j in (0, 1):
                                    b0 = tt * bpt + j * bpb
                                    for q in (0, 1):
                                        nc.tensor.matmul(
                                            pt[q * 64 : (q + 1) * 64,
                                               j * 512 : (j + 1) * 512],
                                            wlt[:, ln * 128 + x * 64 :
                                                ln * 128 + (x + 1) * 64],
                                            cv[:, lpn, b0 : b0 + bpb, x::2, q::2],
                                            start=(x == 0), stop=(x == 1),
                                            skip_group_check=True,
                                            tile_position=(0, q * 64),
                                        )
                            evict(nv[:, lon, tt * bpt : (tt + 1) * bpt, :, :],
                                  pt[:], bias_ap(lvl, n))
            else:
                # multiple nodes per tile (npt = 2, 8 or 32)
                wchunks = {}
                for m0 in range(0, n2, npt):
                    pt = ppool.tile([128, 1024], F32, tag="ps",
                                    name=f"p{lvl}_{m0}")
                    for n in range(m0, m0 + npt):
                        if n % WCH == 0:
                            wchunks[n // WCH] = wchunk(lvl, n, min(WCH, n2 - n))
                    # x-outer is only legal when each node's psum region is a
                    # full 2KB bank (start=True re-zeroes the whole bank).
                    order = ([(x, n) for x in (0, 1)
                              for n in range(m0, m0 + npt)]
                             if pcols >= 512 else
                             [(x, n) for n in range(m0, m0 + npt)
                              for x in (0, 1)])
                    for x, n in order:
                        if True:
                            ln = n % WCH
                            lt = n - m0
                            cv, lpn = cur_ap(parent(n))
                            for q in (0, 1):
                                nc.tensor.matmul(
                                    pt[q * 64 : (q + 1) * 64,
                                       lt * pcols : (lt + 1) * pcols],
                                    wchunks[n // WCH][:, ln * 128 + x * 64 :
                                        ln * 128 + (x + 1) * 64],
                                    cv[:, lpn, :, x::2, q::2],
                                    start=(x == 0), stop=(x == 1),
                                    skip_group_check=True,
                                    tile_position=(0, q * 64),
                                )
                    nv0, lon0 = nxt_ap(m0)
                    if zb:
                        evict(nv0[:, lon0 : lon0 + npt, :, :, :], pt[:], None)
                    else:
                        for n in range(m0, m0 + npt):
                            nv, lon = nxt_ap(n)
                            lt = n - m0
                            evict(nv[:, lon, :, :, :],
                                  pt[:, lt * pcols : (lt + 1) * pcols],
                                  bias_ap(lvl, n))
            if debug:
                ccols = (n2 * BG * Ho * W2o) // len(otiles)
                for si, t_ in enumerate(otiles):
                    nc.sync.dma_start(
                        dbg[lvl][:, si * ccols : (si + 1) * ccols], t_[:])
            cur_segs = out_segs

        # ---------------- level 6 (node pairs, M=64) + dense, woven ----------------
        # L6 runs in two 16-pair psum tiles; as soon as tile t's features are
        # evicted, dense chunk t's matmuls start while the other L6 half (or
        # the output copy/DMA) proceeds.
        F = fpool.tile([128, 32 * BG], BF16, tag="feats", name="feats")
        Fv = F[:].rearrange("p (pr b) -> p pr b", pr=32)
        curv = cur_segs[0][0]
        t2s = fpool.tile([128, NK * NK * BG], F32, tag="t2s", name="t2s")
        wdts = {}

        def l6_half(t):
            pt6 = ppool.tile([128, 16 * BG], F32, tag="ps", name=f"p6_{t}")
            for g0 in (t * 32, t * 32 + WCH):
                w6t = wchunk(6, g0, WCH)
                if g0 == 16:
                    wdts[0] = wchunk("d", 0, 16)
                elif g0 == 48:
                    wdts[1] = wchunk("d", 16, 16)
                for pr in range(g0 // 2, (g0 + WCH) // 2):
                    lpr = pr - t * 16
                    for half in (0, 1):
                        node = 2 * pr + half
                        ln = node - g0
                        for x in (0, 1):
                            nc.tensor.matmul(
                                pt6[half * 64 : (half + 1) * 64,
                                    lpr * BG : (lpr + 1) * BG],
                                w6t[:, ln * 128 + x * 64 :
                                    ln * 128 + (x + 1) * 64],
                                curv[:, node, :, x, 0],
                                start=(x == 0), stop=(x == 1),
                                skip_group_check=True,
                                tile_position=(0, half * 64),
                            )
            if zflags[6]:
                evict(F[:, t * 256 : (t + 1) * 256], pt6[:], None)
            else:
                for pr in range(t * 16, (t + 1) * 16):
                    evict(Fv[:, pr, :], pt6[:, (pr - t * 16) * BG :
                                             (pr - t * 16 + 1) * BG],
                          bias_ap(6, pr))

        def dense_chunk(t):
            # t2 cols parity-major: (par, pair, b); node = 2*pair + par.
            wdt = wdts[t]
            for par in (0, 1):
                ptd = ppool.tile([128, 16 * BG], F32, tag="ps",
                                 name=f"pd{t}_{par}")
                for lp in range(16):
                    p_ = t * 16 + lp
                    nc.tensor.matmul(
                        ptd[:, lp * BG : (lp + 1) * BG],
                        wdt[par * 64 : (par + 1) * 64,
                            lp * 128 : (lp + 1) * 128],
                        Fv[par * 64 : (par + 1) * 64, p_, :],
                        start=True, stop=True,
                        tile_position=(par * 64, 0),
                    )
                evict_ctr[0] += 1
                dst = t2s[:, par * 512 + t * 256 : par * 512 + (t + 1) * 256]
                if evict_ctr[0] % 2 == 0:
                    nc.scalar.copy(dst, ptd[:])
                else:
                    nc.vector.tensor_copy(dst, ptd[:])
                nc.gpsimd.dma_start(
                    t2[:, par * 512 + t * 256 : par * 512 + (t + 1) * 256], dst
                )

        l6_half(0)
        l6_half(1)
        dense_chunk(0)
        dense_chunk(1)
        if debug:
            nc.sync.dma_start(dbg["F"][:], F[:])
    nc.compile()
    return nc


# ----------------------------------------------------------------------------
# entry point
# ----------------------------------------------------------------------------

def kernel(**inputs):
    inputs = {k: np.asarray(v) for k, v in inputs.items()}
    wblobs, zflags = _prep_weights(inputs)
    nc = _build_kernel(zflags)
    in_maps = []
    for c in range(NCORES):
        m = dict(wblobs)
        m["a0"] = _prep_input(inputs["in_data"][c * BC : (c + 1) * BC])
        in_maps.append(m)
    res = run_bass_kernel_spmd(nc, in_maps, list(range(NCORES)))
    outs = [_decode_output(res.results[c]["t2"]) for c in range(NCORES)]
    return np.concatenate(outs, axis=0).astype(np.float32)


if __name__ == "__main__":
    import reference as ref

    inputs = {k: np.asarray(v) for k, v in ref.setup_inputs().items()}
    expected = np.asarray(ref.reference(**inputs))
    actual = kernel(**inputs)
    err = np.abs(actual - expected).max()
    rel = err / np.abs(expected).max()
    print("absmax:", err, "rel:", rel)


# revision 49
# speedup vs baseline: 1.0050x; 1.0050x over previous
"""Trainium2 Bass kernel for nn_ButterflyLayer2D (butterfly 2D CNN).

Strategy: pure data parallel over 8 NeuronCores (16 batch each).

Layout invariant: activations live in SBUF as [128 = (w%2)*64 + c,
(node, b, h, w//2)] bf16.  Each per-node 2x2-stride-2 conv is then a set
of matmuls: contraction K=128=(dw,c), accumulated over x=dh (start/stop),
with output w-parity q selected by slicing the rhs w2 dim (q::2) and
writing psum rows q*64+c via col tile_position (0, q*64).  The psum rows
(q, c_out) are exactly the next level's partition layout, so every
eviction is a contiguous full-width relu+bias op.

PSUM tiles are [128, 1024] (2 banks, ring of 4).  Within a tile the
matmuls are ordered x-outer so one weight image (w[n,x] in both column
halves) serves 2 consecutive 512-col streams.  Each tile is drained by
ONE [128, 1024] relu+bias op (the empirically fastest evict shape,
~730ns), alternating between ScalarE and VectorE per tile.  X is split
into 8 bl-pair tiles so L1 matmuls depend only on the input-conv
evictions of their own chunk (tile-granular dependency tracking).

When a level's bias is nonzero, multi-node tiles fall back to per-node
evictions (bias must be per-partition-constant within one evict op).
"""

import numpy as np
from contextlib import ExitStack

import concourse.bass as bass
import concourse.tile as tile
from concourse import bacc, mybir
from concourse.bass_utils import run_bass_kernel_spmd

F32 = mybir.dt.float32
BF16 = mybir.dt.bfloat16
AF = mybir.ActivationFunctionType
ALU = mybir.AluOpType

B, IN, NLVL, KLVL, C = 128, 256, 6, 3, 64
NK, OU, OV = 8, 8, 8
NCORES = 8
BC = B // NCORES          # 16 per-core batch
BG = BC
HALF = BG // 4
LVL_NODES = [4, 16, 64, 64, 64, 64]
LVL_HIN = [64, 32, 16, 8, 4, 2]
WCH = 16                  # weight streaming chunk (nodes)
BIAS_OFF = {0: 0, 1: 1, 2: 5, 3: 21, 4: 85, 5: 149, 6: 213}  # cols in bias blob


# ----------------------------------------------------------------------------
# host-side pre-arrangement
# ----------------------------------------------------------------------------

def _prep_weights(inputs):
    """Weights/biases blobs shared by all cores."""
    import ml_dtypes

    out = {}
    zflags = {}
    # fin2 [128=(G,p,qp,yp), 128=(q,c)]: W = fin[p,qp,c] iff yp==q, else 0.
    # K=32 per batch-group G; the zero rows select output w-parity q so one
    # M=128 matmul produces both parity halves of the psum tile.
    fin = inputs["in_filter"][:, :, 0, :].astype(np.float32)  # [p, qp, c]
    f2 = np.zeros((4, 4, 2, 2, C), np.float32)                # [p,qp,yp,q,c]
    for q in range(2):
        f2[:, :, q, q, :] = fin
    f2 = f2.reshape(32, 2 * C)
    finr = np.zeros((128, 2 * C), np.float32)
    for g in range(4):
        finr[g * 32 : (g + 1) * 32] = f2
    out["fin"] = finr.astype(ml_dtypes.bfloat16)

    bias_cols = [np.concatenate([inputs["in_bias"], inputs["in_bias"]])
                 .reshape(128, 1).astype(np.float32)]
    for lvl in range(1, NLVL + 1):
        f = inputs[f"f{lvl}"].astype(np.float32)  # [n,n,2,2,C,C] (x=dh,y=dw,ci,co)
        n = f.shape[0]
        assert n == 2 ** min(lvl, KLVL)
        w = f.transpose(0, 1, 3, 4, 2, 5).reshape(n * n, 2 * C, 2 * C)
        out[f"w{lvl}"] = np.ascontiguousarray(w.transpose(1, 0, 2)).reshape(
            128, n * n * 128
        ).astype(ml_dtypes.bfloat16)
        b = inputs[f"b{lvl}"].astype(np.float32).reshape(n * n, C)
        zflags[lvl] = not np.any(b)
        if lvl < NLVL:
            bb = np.concatenate([b, b], axis=1)  # rows (q,c), dup across q
            bias_cols.append(np.ascontiguousarray(bb.T))
        else:
            bb = b.reshape(n * n // 2, 2 * C)    # rows (cEven,cOdd) per pair
            bias_cols.append(np.ascontiguousarray(bb.T))
    out["bb"] = np.concatenate(bias_cols, axis=1)  # [128, 245]

    # dense, stacked pairs: rows 0:64 = c for even node, 64:128 = odd;
    # cols (pair, (r,ou,ov))
    wd = inputs["Wd"].astype(np.float32).reshape(NK * NK, 2, C, OU * OV)
    wdn = wd.transpose(0, 2, 1, 3).reshape(NK * NK, C, 2 * OU * OV)
    wds = np.zeros((128, (NK * NK // 2) * 2 * OU * OV), np.float32)
    for p in range(NK * NK // 2):
        wds[0:64, p * 128 : (p + 1) * 128] = wdn[2 * p]
        wds[64:128, p * 128 : (p + 1) * 128] = wdn[2 * p + 1]
    out["wd"] = np.ascontiguousarray(wds).astype(ml_dtypes.bfloat16)
    return out, zflags


def _prep_input(in_data_core):
    """Per-core input blob [128 = (b//4)*32 + (i%4)*8 + (j%4)*2 + (j//4)%2,
    (b%4, x=i//4, y2=j//8)] — all 128 partitions so the staging DMA runs
    at full width; column chunks (by b%4) land in consumption order."""
    import ml_dtypes

    ind = in_data_core[:, :, :, 0]  # [16, 256, 256]
    a = ind.reshape(4, 4, 64, 4, 32, 2, 4)      # [G, m, x, p, y2, yp, qp]
    a = a.transpose(0, 3, 6, 5, 1, 2, 4)        # [G, p, qp, yp, m, x, y2]
    return np.ascontiguousarray(a).reshape(128, 4 * 64 * 32).astype(
        ml_dtypes.bfloat16
    )


def _decode_output(t2_core):
    """t2 [128=(r,ou,ov), (par, pair, bl)], node=2*pair+par -> [16, 64, 64, 2]."""
    t = t2_core.reshape(2, OU, OV, 2, 32, BG)       # r,ou,ov,par,pair,bl
    t = t.transpose(0, 1, 2, 4, 3, 5)               # r,ou,ov,pair,par,bl
    t = t.reshape(2, OU, OV, NK, NK, BG)            # r,ou,ov,u,v,bl
    t = t.transpose(5, 3, 1, 4, 2, 0)               # bl,u,ou,v,ov,r
    return np.ascontiguousarray(t).reshape(BG, NK * OU, NK * OV, 2)


# ----------------------------------------------------------------------------
# device kernel
# ----------------------------------------------------------------------------

def _build_kernel(zflags, debug=False):
    nc = bacc.Bacc(None, target_bir_lowering=False)
    p = {}
    p["a0"] = nc.declare_dram_parameter("a0", [128, 4 * 64 * 32], BF16, isOutput=False)
    p["fin"] = nc.declare_dram_parameter("fin", [128, 2 * C], BF16, isOutput=False)
    p["bb"] = nc.declare_dram_parameter("bb", [128, 245], F32, isOutput=False)
    for lvl in range(1, NLVL + 1):
        n2 = LVL_NODES[lvl - 1]
        p[f"w{lvl}"] = nc.declare_dram_parameter(f"w{lvl}", [128, n2 * 128], BF16, isOutput=False)
    p["wd"] = nc.declare_dram_parameter("wd", [128, 32 * 128], BF16, isOutput=False)
    t2 = nc.declare_dram_parameter("t2", [128, NK * NK * BG], F32, isOutput=True)
    dbg = {}
    if debug:
        dbg["X"] = nc.declare_dram_parameter("dbgX", [128, BG * 64 * 32], BF16, isOutput=True)
        for lvl in range(1, 6):
            n2 = LVL_NODES[lvl - 1]
            Ho = LVL_HIN[lvl - 1] // 2
            dbg[lvl] = nc.declare_dram_parameter(
                f"dbgL{lvl}", [128, n2 * BG * Ho * max(Ho // 2, 1)], BF16, isOutput=True)
        dbg["F"] = nc.declare_dram_parameter("dbgF", [128, 32 * BG], BF16, isOutput=True)

    evict_ctr = [0]

    with tile.TileContext(nc) as tc, ExitStack() as ctx:
        const = ctx.enter_context(tc.tile_pool(name="const", bufs=1))
        wpool = ctx.enter_context(tc.tile_pool(name="wts", bufs=8))
        apool = ctx.enter_context(tc.tile_pool(name="acts", bufs=1))
        inpool = ctx.enter_context(tc.tile_pool(name="inp", bufs=1))
        fpool = ctx.enter_context(tc.tile_pool(name="feat", bufs=1))
        ppool = ctx.enter_context(tc.tile_pool(name="ps", bufs=4, space="PSUM"))

        # --- startup DMAs: fin, input chunks, bias blob, then weights ---
        fin_t = const.tile([128, 2 * C], BF16)
        nc.sync.dma_start(fin_t[:], p["fin"][:])
        a0m = [inpool.tile([128, 64 * 32], BF16, tag=f"a0s{m}", name=f"a0s{m}")
               for m in range(4)]
        nc.gpsimd.dma_start(a0m[0][:, 0:1024], p["a0"][:, 0:1024])
        bb_t = const.tile([128, 245], F32, tag="bb", name="bb")
        nc.sync.dma_start(bb_t[:], p["bb"][:])
        nc.gpsimd.dma_start(a0m[0][:, 1024:2048], p["a0"][:, 1024:2048])
        for m in range(1, 4):
            for h in range(2):
                nc.gpsimd.dma_start(
                    a0m[m][:, h * 1024 : (h + 1) * 1024],
                    p["a0"][:, m * 2048 + h * 1024 : m * 2048 + (h + 1) * 1024],
                )
        a0v = [t_[:].rearrange("p (x y) -> p x y", x=64) for t_ in a0m]

        # PE warm-up: dummy matmuls on an uninitialized junk tile (no DMA
        # dependency) into a dedicated 1-bank psum tile nobody reads, so the
        # HAM clock gate reaches 8/8 before real work starts and stays there
        # through the input-chunk staircase.
        junk = const.tile([128, 128], BF16, tag="junk", name="junk")
        nc.vector.memset(junk[:], 1.0)
        warm = ppool.tile([128, 128], F32, tag="ps", name="warm")

        def dummies(k):
            for _ in range(k):
                nc.tensor.matmul(
                    warm[:], junk[:], junk[:], start=True, stop=True,
                )

        dummies(40)

        def bias_ap(lvl, n):
            off = BIAS_OFF[lvl] + n
            return bb_t[:, off : off + 1]

        def evict(out_ap, psum_ap, b_ap):
            """relu(psum + bias) -> sbuf, alternating engines per op."""
            evict_ctr[0] += 1
            if evict_ctr[0] % 2 == 0:
                if b_ap is None:
                    nc.scalar.activation(out_ap, psum_ap, AF.Relu)
                else:
                    nc.scalar.activation(out_ap, psum_ap, AF.Relu, bias=b_ap)
            else:
                if b_ap is None:
                    nc.vector.tensor_scalar(out_ap, psum_ap, 0.0, None, op0=ALU.max)
                else:
                    nc.vector.tensor_scalar(out_ap, psum_ap, b_ap, 0.0,
                                            op0=ALU.add, op1=ALU.max)

        def wchunk(lvl, g0, gn):
            wlt = wpool.tile([128, WCH * 128], BF16, tag="wch", name=f"w{lvl}_{g0}")
            src = p["wd"] if lvl == "d" else p[f"w{lvl}"]
            nc.gpsimd.dma_start(
                wlt[:, : gn * 128], src[:, g0 * 128 : (g0 + gn) * 128]
            )
            return wlt

        # ---------------- input conv + L1, interleaved ----------------
        # X split into 8 bl-pair tiles: [128=(w%2,c), (bl2, h=64, w2=32)]
        Xtile, Xt = [], []
        for u in range(8):
            t_ = apool.tile([128, 2 * 64 * 32], BF16, tag=f"sA{u}", name=f"x{u}")
            Xtile.append(t_)
            Xt.append(t_[:].rearrange("p (b h w) -> p b h w", b=2, h=64))
        L1n = LVL_NODES[0]
        L1out = apool.tile([128, L1n * BG * 32 * 16], BF16, tag="sB", name="a1")
        L1v = L1out[:].rearrange("p (n b h w) -> p n b h w", n=L1n, b=BG, h=32)
        w1t = wchunk(1, 0, L1n)

        def input_bl(bl):
            g, m = bl // 4, bl % 4
            for th in (0, 1):
                pt = ppool.tile([128, 1024], F32, tag="ps", name=f"pin{bl}_{th}")
                for j in (0, 1):
                    xq = th * 2 + j
                    nc.tensor.matmul(
                        pt[:, j * 512 : (j + 1) * 512],
                        fin_t[g * 32 : (g + 1) * 32, :],
                        a0v[m][g * 32 : (g + 1) * 32,
                               xq * 16 : (xq + 1) * 16, :],
                        start=True, stop=True,
                        tile_position=(g * 32, 0),
                    )
                evict(Xt[bl // 2][:, bl % 2, th * 32 : (th + 1) * 32, :],
                      pt[:], bias_ap(0, 0))

        def l1_pair(n, ua, ub):
            # one node, two bl-pair tiles, x-outer across both (one weight
            # image per x-phase serves 4 streams)
            pts = {ua: ppool.tile([128, 1024], F32, tag="ps", name=f"p1_{n}_{ua}"),
                   ub: ppool.tile([128, 1024], F32, tag="ps", name=f"p1_{n}_{ub}")}
            for x in (0, 1):
                for u in (ua, ub):
                    for j in (0, 1):
                        for q in (0, 1):
                            nc.tensor.matmul(
                                pts[u][q * 64 : (q + 1) * 64,
                                       j * 512 : (j + 1) * 512],
                                w1t[:, n * 128 + x * 64 : n * 128 + (x + 1) * 64],
                                Xt[u][:, j, x::2, q::2],
                                start=(x == 0), stop=(x == 1),
                                skip_group_check=True,
                                tile_position=(0, q * 64),
                            )
            for u in (ua, ub):
                evict(L1v[:, n, 2 * u : 2 * u + 2, :, :], pts[u][:],
                      bias_ap(1, n))

        for G in range(4):
            input_bl(G * 4)
        for G in range(4):
            input_bl(G * 4 + 1)
        for G in range(4):
            input_bl(G * 4 + 2)
            l1_pair(G, 0, 2)
        for G in range(4):
            input_bl(G * 4 + 3)
            l1_pair(G, 4, 6)
        for n in range(L1n):
            l1_pair(n, 1, 3)
        for n in range(L1n):
            l1_pair(n, 5, 7)
        if debug:
            for u in range(8):
                nc.sync.dma_start(
                    dbg["X"][:, u * 4096 : (u + 1) * 4096], Xtile[u][:]
                )
            nc.sync.dma_start(dbg[1][:], L1out[:])

        # ---------------- levels 2..5 ----------------
        # L2 output is segmented into 8 node-pair tiles reusing the X chunk
        # tags (finer deps + SBUF reuse); L3 reuses sB after L1out dies;
        # L4/L5 outputs reuse sA0/sA1.
        cur_segs = [(L1v, L1n)]
        for lvl in range(2, 6):
            n2 = LVL_NODES[lvl - 1]
            grid = int(np.sqrt(n2))
            Hin = LVL_HIN[lvl - 1]
            Ho, W2o = Hin // 2, Hin // 4
            pcols = BG * Ho * W2o               # output cols per node
            npt = max(1, 1024 // pcols)         # nodes per psum tile
            zb = zflags[lvl]
            pgrid = int(np.sqrt(LVL_NODES[lvl - 2] if lvl > 2 else 1))
            pgrid = int(np.sqrt(cur_segs[0][1] * len(cur_segs)))

            def parent(n):
                if lvl <= KLVL:
                    return (n // grid // 2) * pgrid + (n % grid) // 2
                return n

            def cur_ap(pn):
                nps = cur_segs[0][1]
                return cur_segs[pn // nps][0], pn % nps

            if lvl == 2:
                otiles = [apool.tile([128, 2 * BG * Ho * W2o], BF16,
                                     tag=f"sA{u}", name=f"a2_{u}")
                          for u in range(8)]
                out_segs = [(t_[:].rearrange("p (n b h w) -> p n b h w",
                                             n=2, b=BG, h=Ho), 2)
                            for t_ in otiles]
            else:
                tag = {3: "sB", 4: "sA0", 5: "sA1"}[lvl]
                ot = apool.tile([128, n2 * BG * Ho * W2o], BF16,
                                tag=tag, name=f"a{lvl}")
                otiles = [ot]
                out_segs = [(ot[:].rearrange("p (n b h w) -> p n b h w",
                                             n=n2, b=BG, h=Ho), n2)]

            def nxt_ap(n):
                nps = out_segs[0][1]
                return out_segs[n // nps][0], n % nps

            if pcols >= 1024:
                # one or more [128,1024] tiles per node; 512-col b-blocks
                tpn = pcols // 1024             # tiles per node
                bpt = BG // tpn                 # b per tile
                bpb = max(1, bpt // 2)          # b per 512-block
                for g0 in range(0, n2, WCH):
                    wlt = wchunk(lvl, g0, min(WCH, n2 - g0))
                    for n in range(g0, g0 + min(WCH, n2 - g0)):
                        ln = n - g0
                        cv, lpn = cur_ap(parent(n))
                        nv, lon = nxt_ap(n)
                        for tt in range(tpn):
                            pt = ppool.tile([128, 1024], F32, tag="ps",
                                            name=f"p{lvl}_{n}_{tt}")
                            for x in (0, 1):
                                for j in (0, 1):
                                    b0 = tt * bpt + j * bpb
                                    for q in (0, 1):
                                        nc.tensor.matmul(
                                            pt[q * 64 : (q + 1) * 64,
                                               j * 512 : (j + 1) * 512],
                                            wlt[:, ln * 128 + x * 64 :
                                                ln * 128 + (x + 1) * 64],
                                            cv[:, lpn, b0 : b0 + bpb, x::2, q::2],
                                            start=(x == 0), stop=(x == 1),
                                            skip_group_check=True,
                                            tile_position=(0, q * 64),
                                        )
                            evict(nv[:, lon, tt * bpt : (tt + 1) * bpt, :, :],
                                  pt[:], bias_ap(lvl, n))
            else:
                # multiple nodes per tile (npt = 2, 8 or 32)
                wchunks = {}
                for m0 in range(0, n2, npt):
                    pt = ppool.tile([128, 1024], F32, tag="ps",
                                    name=f"p{lvl}_{m0}")
                    for n in range(m0, m0 + npt):
                        if n % WCH == 0:
                            wchunks[n // WCH] = wchunk(lvl, n, min(WCH, n2 - n))
                    # x-outer is only legal when each node's psum region is a
                    # full 2KB bank (start=True re-zeroes the whole bank).
                    order = ([(x, n) for x in (0, 1)
                              for n in range(m0, m0 + npt)]
                             if pcols >= 512 else
                             [(x, n) for n in range(m0, m0 + npt)
                              for x in (0, 1)])
                    for x, n in order:
                        if True:
                            ln = n % WCH
                            lt = n - m0
                            cv, lpn = cur_ap(parent(n))
                            for q in (0, 1):
                                nc.tensor.matmul(
                                    pt[q * 64 : (q + 1) * 64,
                                       lt * pcols : (lt + 1) * pcols],
                                    wchunks[n // WCH][:, ln * 128 + x * 64 :
                                        ln * 128 + (x + 1) * 64],
                                    cv[:, lpn, :, x::2, q::2],
                                    start=(x == 0), stop=(x == 1),
                                    skip_group_check=True,
                                    tile_position=(0, q * 64),
                                )
                    nv0, lon0 = nxt_ap(m0)
                    if zb:
                        evict(nv0[:, lon0 : lon0 + npt, :, :, :], pt[:], None)
                    else:
                        for n in range(m0, m0 + npt):
                            nv, lon = nxt_ap(n)
                            lt = n - m0
                            evict(nv[:, lon, :, :, :],
                                  pt[:, lt * pcols : (lt + 1) * pcols],
                                  bias_ap(lvl, n))
            if debug:
                ccols = (n2 * BG * Ho * W2o) // len(otiles)
                for si, t_ in enumerate(otiles):
                    nc.sync.dma_start(
                        dbg[lvl][:, si * ccols : (si + 1) * ccols], t_[:])
            cur_segs = out_segs

        # ---------------- level 6 (node pairs, M=64) + dense, woven ----------------
        # L6 runs in two 16-pair psum tiles; as soon as tile t's features are
        # evicted, dense chunk t's matmuls start while the other L6 half (or
        # the output copy/DMA) proceeds.
        F = fpool.tile([128, 32 * BG], BF16, tag="feats", name="feats")
        Fv = F[:].rearrange("p (pr b) -> p pr b", pr=32)
        curv = cur_segs[0][0]
        t2s = fpool.tile([128, NK * NK * BG], F32, tag="t2s", name="t2s")
        wdts = {}

        def l6_half(t):
            pt6 = ppool.tile([128, 16 * BG], F32, tag="ps", name=f"p6_{t}")
            for g0 in (t * 32, t * 32 + WCH):
                w6t = wchunk(6, g0, WCH)
                if g0 == 16:
                    wdts[0] = wchunk("d", 0, 16)
                elif g0 == 48:
                    wdts[1] = wchunk("d", 16, 16)
                for pr in range(g0 // 2, (g0 + WCH) // 2):
                    lpr = pr - t * 16
                    for half in (0, 1):
                        node = 2 * pr + half
                        ln = node - g0
                        for x in (0, 1):
                            nc.tensor.matmul(
                                pt6[half * 64 : (half + 1) * 64,
                                    lpr * BG : (lpr + 1) * BG],
                                w6t[:, ln * 128 + x * 64 :
                                    ln * 128 + (x + 1) * 64],
                                curv[:, node, :, x, 0],
                                start=(x == 0), stop=(x == 1),
                                skip_group_check=True,
                                tile_position=(0, half * 64),
                            )
            if zflags[6]:
                evict(F[:, t * 256 : (t + 1) * 256], pt6[:], None)
            else:
                for pr in range(t * 16, (t + 1) * 16):
                    evict(Fv[:, pr, :], pt6[:, (pr - t * 16) * BG :
                                             (pr - t * 16 + 1) * BG],
                          bias_ap(6, pr))

        def dense_chunk(t):
            # t2 cols parity-major: (par, pair, b); node = 2*pair + par.
            wdt = wdts[t]
            for par in (0, 1):
                ptd = ppool.tile([128, 16 * BG], F32, tag="ps",
                                 name=f"pd{t}_{par}")
                for lp in range(16):
                    p_ = t * 16 + lp
                    nc.tensor.matmul(
                        ptd[:, lp * BG : (lp + 1) * BG],
                        wdt[par * 64 : (par + 1) * 64,
                            lp * 128 : (lp + 1) * 128],
                        Fv[par * 64 : (par + 1) * 64, p_, :],
                        start=True, stop=True,
                        tile_position=(par * 64, 0),
                    )
                evict_ctr[0] += 1
                dst = t2s[:, par * 512 + t * 256 : par * 512 + (t + 1) * 256]
                if evict_ctr[0] % 2 == 0:
                    nc.scalar.copy(dst, ptd[:])
                else:
                    nc.vector.tensor_copy(dst, ptd[:])
                nc.sync.dma_start(
                    t2[:, par * 512 + t * 256 : par * 512 + (t + 1) * 256], dst
                )

        l6_half(0)
        l6_half(1)
        dense_chunk(0)
        dense_chunk(1)
        if debug:
            nc.sync.dma_start(dbg["F"][:], F[:])
    nc.compile()
    return nc


# ----------------------------------------------------------------------------
# entry point
# ----------------------------------------------------------------------------

def kernel(**inputs):
    inputs = {k: np.asarray(v) for k, v in inputs.items()}
    wblobs, zflags = _prep_weights(inputs)
    nc = _build_kernel(zflags)
    in_maps = []
    for c in range(NCORES):
        m = dict(wblobs)
        m["a0"] = _prep_input(inputs["in_data"][c * BC : (c + 1) * BC])
        in_maps.append(m)
    res = run_bass_kernel_spmd(nc, in_maps, list(range(NCORES)))
    outs = [_decode_output(res.results[c]["t2"]) for c in range(NCORES)]
    return np.concatenate(outs, axis=0).astype(np.float32)


if __name__ == "__main__":
    import reference as ref

    inputs = {k: np.asarray(v) for k, v in ref.setup_inputs().items()}
    expected = np.asarray(ref.reference(**inputs))
    actual = kernel(**inputs)
    err = np.abs(actual - expected).max()
    rel = err / np.abs(expected).max()
    print("absmax:", err, "rel:", rel)
